# revision 90
# baseline (speedup 1.0000x reference)
"""DGCRN Trainium2 Bass kernel (restructured).

Problem: nn_DGCRN_67327907332247 (B=32, T=12, N=512, DIN=2, HID=64, CHEB_K=3,
EMB=10, DOUT=1, YCOV=1). Data-parallel over batch: 8 cores x 4 batches each.

Design (v2 — "v-projection" formulation, batch-stage interleaving):
 - State feature-major: inp [66,512] = [h(0:64); xt/go,ycov(64:66)].
 - Per gate path, exploit linearity G_k(x) W_k = (x W_k) pre-projected:
     v_j = inp_nm[j] @ [W1|W2]   (4 matmuls, moving dim 256, node-major out)
     zr^T = W0^T@inp + sum_j v1_j^T@A^T_j + sum_j v2_j^T@A2^T_j  (9 mm, 512)
   This kills all per-cell PE transposes and the xg PSUM->SBUF round trips.
 - z/r gates fused into one [128,512] accumulation + one sigmoid (bias is the
   stacked [gbz;gbr]).
 - Work for the 4 local batches is emitted stage-interleaved (vb0 vb1 zr0 vb2
   zr1 ...) so the PE never drains; a continuously-busy PE ramps from 1.2GHz
   to 2.4GHz (p-state) which alone is ~2x on matmul time.
 - All matmuls f32r with moving dims >= 256 (1 cycle/row). Producers of
   matmul operands write through .bitcast(f32r) APs (BIR rounded-producer
   rule); DMA loads are staged + round-copied.
 - Supports: A^T / (2A^2-I)^T tiles SBUF-resident as in v1; softmax skips
   row-max (relu-bounded); -I handled by subtracting the [128,128] identity
   from the diagonal block (no idsl slabs).
 - Elementwise work spread across Scalar(ACT)/Vector(DVE)/GpSimd(Pool) to
   stay under the PE's per-cell time.
"""

import numpy as np

B = 32
NCORES = 8
BL = B // NCORES  # 4 local batches
T = 12
N = 512
NT = N // 128  # 4 node tiles
DIN = 2
HID = 64
EMB = 10
CIN = DIN + HID  # 66
K = 3

_CACHE = {}


def _build_module():
    import concourse.bacc as bacc
    import concourse.mybir as mybir
    from concourse import masks, tile

    f32 = mybir.dt.float32
    Act = mybir.ActivationFunctionType
    f32r = mybir.dt.float32r

    nc = bacc.Bacc("TRN2", target_bir_lowering=False, debug=False)

    def mm(out, lhsT, rhs, **kw):
        nc.tensor.matmul(out, lhsT.bitcast(f32r), rhs.bitcast(f32r), **kw)

    def rr(ap):
        return ap.bitcast(f32r)

    # x/y are fed pre-transposed (feature-major rows) by _in_maps so the
    # per-step loads are contiguous DMAs instead of 512-way gathers.
    x_d = nc.dram_tensor("x", [BL, T, DIN, N], f32, kind="ExternalInput").ap()
    y_d = nc.dram_tensor("y_cov", [BL, T, 1, N], f32, kind="ExternalInput").ap()
    ne_d = nc.dram_tensor("node_emb", [N, EMB], f32, kind="ExternalInput").ap()
    egw_d = nc.dram_tensor("enc_gW", [K * CIN, 2 * HID], f32, kind="ExternalInput").ap()
    egb_d = nc.dram_tensor("enc_gb", [2 * HID], f32, kind="ExternalInput").ap()
    euw_d = nc.dram_tensor("enc_uW", [K * CIN, HID], f32, kind="ExternalInput").ap()
    eub_d = nc.dram_tensor("enc_ub", [HID], f32, kind="ExternalInput").ap()
    dgw_d = nc.dram_tensor("dec_gW", [K * CIN, 2 * HID], f32, kind="ExternalInput").ap()
    dgb_d = nc.dram_tensor("dec_gb", [2 * HID], f32, kind="ExternalInput").ap()
    duw_d = nc.dram_tensor("dec_uW", [K * CIN, HID], f32, kind="ExternalInput").ap()
    dub_d = nc.dram_tensor("dec_ub", [HID], f32, kind="ExternalInput").ap()
    pw_d = nc.dram_tensor("proj_W", [HID, 1], f32, kind="ExternalInput").ap()
    pb_d = nc.dram_tensor("proj_b", [1], f32, kind="ExternalInput").ap()
    hw_d = nc.dram_tensor("hyper_W", [HID, EMB], f32, kind="ExternalInput").ap()
    hb_d = nc.dram_tensor("hyper_b", [EMB], f32, kind="ExternalInput").ap()
    out_d = nc.dram_tensor("out", [BL, T, N, 1], f32, kind="ExternalOutput").ap()

    with tile.TileContext(nc) as tc:
        with (
            tc.tile_pool(name="const", bufs=1) as cp,
            tc.tile_pool(name="state", bufs=1) as sp,
            tc.tile_pool(name="work", bufs=2) as wp,
            tc.tile_pool(name="psum", bufs=1, space="PSUM") as pp,
        ):
            ident = cp.tile([128, 128], f32)
            masks.make_identity(nc, ident[:])

            def rdma(dst, src):
                # DMA whose dest AP is f32r-bitcast: satisfies the BIR
                # rounded-producer rule without a staging round-copy.
                nc.sync.dma_start(dst.bitcast(f32r), src.bitcast(f32r))

            # node-embedding + x0 loads first on the Sync queue: the encoder
            # support build and first cells (startup critical path) wait on
            # these; weight DMAs have more slack.
            ne_nm = []
            for i in range(NT):
                t = wp.tile([128, EMB], f32, name=f"ne_nm{i}", tag="ne_nm", bufs=NT)
                nc.sync.dma_start(t[:], ne_d[i * 128 : (i + 1) * 128, :])
                ne_nm.append(t)

            # ---- persistent per-batch state ---------------------------------
            zrow = wp.tile([CIN, N], f32, tag="wstg3")
            nc.vector.memset(zrow[:], 0.0)
            inp_pp = []
            cand_t = []
            for b in range(BL):
                pair = []
                for p in range(2):
                    it = sp.tile([CIN, N], f32, name=f"inp{b}_{p}")
                    if p == 0:
                        nc.vector.tensor_copy(rr(it[0:HID, :]), zrow[0:HID, :])
                    pair.append(it)
                inp_pp.append(pair)
                ct = sp.tile([CIN, N], f32, name=f"cand{b}")
                cand_t.append(ct)
            # prime xt for t=0 (DMA straight into state rows 64:66)
            for b in range(BL):
                rdma(inp_pp[b][0][HID:CIN, :], x_d[b, 0])

            def acc_tile(name, p=128, n=N):
                return pp.tile([p, n], f32, name=name, tag="acc", bufs=7)

            # ---- weights ----------------------------------------------------
            # feature layout [h(0:64); xt(64:66)] -> permute W rows.
            def wload(dst, src, k, c0, cols):
                rdma(dst[0:HID, c0 : c0 + cols], src[k * CIN + DIN : (k + 1) * CIN, :])
                rdma(dst[HID:CIN, c0 : c0 + cols], src[k * CIN : k * CIN + DIN, :])

            gw0_e = cp.tile([CIN, 2 * HID], f32)
            gws_e = cp.tile([CIN, 4 * HID], f32)
            uw0_e = cp.tile([CIN, HID], f32)
            uws_e = cp.tile([CIN, 4 * HID], f32)
            gw0_d = cp.tile([CIN, 2 * HID], f32)
            gws_d = cp.tile([CIN, 4 * HID], f32)
            uw0_d = cp.tile([CIN, HID], f32)
            uws_d = cp.tile([CIN, 4 * HID], f32)
            wload(gw0_e, egw_d, 0, 0, 2 * HID)
            wload(gws_e, egw_d, 1, 0, 2 * HID)
            wload(gws_e, egw_d, 2, 2 * HID, 2 * HID)
            # identity slabs: zeros except I at column block j (f32r-produced)
            zfull = wp.tile([128, N], f32, tag="wstg4")
            nc.vector.memset(zfull[:], 0.0)
            idsl = []
            for j in range(NT):
                s = cp.tile([128, N], f32, name=f"idsl{j}")
                nc.vector.tensor_copy(rr(s[:]), zfull[:])
                nc.vector.tensor_copy(rr(s[:, j * 128 : (j + 1) * 128]), ident[:])
                idsl.append(s)
            wload(uw0_e, euw_d, 0, 0, HID)
            wload(uws_e, euw_d, 1, 0, HID)
            wload(uws_e, euw_d, 2, HID, HID)
            wload(uws_e, euw_d, 0, 2 * HID, HID)  # uW0 via identity slabs (enc)
            wload(gw0_d, dgw_d, 0, 0, 2 * HID)
            wload(gws_d, dgw_d, 1, 0, 2 * HID)
            wload(gws_d, dgw_d, 2, 2 * HID, 2 * HID)
            wload(uw0_d, duw_d, 0, 0, HID)
            wload(uws_d, duw_d, 1, 0, HID)
            wload(uws_d, duw_d, 2, HID, HID)
            # zero-pad tails of the uW stacks (keeps moving dim 256)
            zpad = wp.tile([CIN, 2 * HID], f32, tag="wstg")
            nc.vector.memset(zpad[:], 0.0)
            nc.scalar.copy(rr(uws_e[:, 3 * HID : 4 * HID]), zpad[:, 0:HID])
            nc.scalar.copy(rr(uws_d[:, 2 * HID : 4 * HID]), zpad[:])

            def bias_tile(name, src, n):
                t = cp.tile([n, 1], f32, name=name)
                nc.sync.dma_start(t[:], src.rearrange("(p o) -> p o", o=1))
                return t

            gbz_e = bias_tile("gbz_e", egb_d[0:HID], HID)
            gbr_e = bias_tile("gbr_e", egb_d[HID : 2 * HID], HID)
            ub_e = bias_tile("ub_e", eub_d, HID)
            gbz_d = bias_tile("gbz_d", dgb_d[0:HID], HID)
            gbr_d = bias_tile("gbr_d", dgb_d[HID : 2 * HID], HID)
            ub_d = bias_tile("ub_d", dub_d, HID)
            pb = bias_tile("pb", pb_d, 1)
            hb = bias_tile("hb", hb_d, EMB)
            pw = cp.tile([HID, 1], f32)
            rdma(pw[:], pw_d[:, :])
            hwt = cp.tile([HID, EMB], f32)
            rdma(hwt[:], hw_d[:, :])

            # ---- support construction (n-way interleaved) -------------------
            # mid() is called between the softmax and transpose phases: the
            # PE idles there while DVE/Scalar finish the softmax chains, so
            # support-independent matmuls (the first timestep's vbuilds) can
            # fill the hole.
            def build_supports(builds, tag, mid=None):
                # builds: list of (emit_scores, at_tiles, at2_tiles)
                anm = {}
                for i in range(NT):
                    for bi, (es, _, _) in enumerate(builds):
                        ps = es(i)
                        nc.vector.tensor_scalar_max(ps[:], ps[:], 0.0)
                        ex = wp.tile(
                            [128, N], f32, name=f"ex_{tag}{bi}_{i}", tag="anm", bufs=8
                        )
                        esum = wp.tile(
                            [128, 1], f32, name=f"es_{tag}{bi}_{i}", tag="esum", bufs=4
                        )
                        nc.scalar.activation(
                            rr(ex[:]), ps[:], Act.Exp, accum_out=esum[:]
                        )
                        rinv = wp.tile(
                            [128, 1], f32, name=f"ri_{tag}{bi}_{i}", tag="rinv", bufs=4
                        )
                        nc.vector.reciprocal(rinv[:], esum[:])
                        nc.vector.tensor_scalar_mul(rr(ex[:]), ex[:], rinv[:])
                        anm[bi, i] = ex
                if mid is not None:
                    mid()
                for bi, (_, at, _) in enumerate(builds):
                    for j in range(NT):
                        ps_t = acc_tile(f"ps_t_{tag}{bi}_{j}")
                        for i in range(NT):
                            nc.tensor.matmul(
                                ps_t[:, i * 128 : (i + 1) * 128],
                                anm[bi, i][:, j * 128 : (j + 1) * 128],
                                ident[:],
                                is_transpose=True,
                                skip_group_check=True,
                            )
                        nc.scalar.copy(rr(at[j][:]), ps_t[:])
                for bi, (_, at, at2) in enumerate(builds):
                    for j in range(NT):
                        ps_c = acc_tile(f"ps_c_{tag}{bi}_{j}")
                        for k in range(NT):
                            mm(
                                ps_c[:],
                                anm[bi, k][:, j * 128 : (j + 1) * 128],
                                at[k][:],
                                start=(k == 0),
                                stop=(k == NT - 1),
                            )
                        nc.scalar.mul(rr(at2[j][:]), ps_c[:], 2.0)
                        nc.vector.tensor_sub(
                            rr(at2[j][:, j * 128 : (j + 1) * 128]),
                            at2[j][:, j * 128 : (j + 1) * 128],
                            ident[:],
                        )

            # ---- encoder support --------------------------------------------
            ps_ne = acc_tile("ps_ne", p=EMB)
            for i in range(NT):
                nc.tensor.matmul(
                    ps_ne[:, i * 128 : (i + 1) * 128],
                    ne_nm[i][:],
                    ident[:],
                    is_transpose=True,
                    skip_group_check=True,
                )
            neT = cp.tile([EMB, N], f32)
            nc.vector.tensor_copy(rr(neT[:]), ps_ne[:])

            aet = [cp.tile([128, N], f32, name=f"aet{j}") for j in range(NT)]
            aet2 = [cp.tile([128, N], f32, name=f"aet2_{j}") for j in range(NT)]

            def enc_scores(i):
                ps = acc_tile(f"ps_enc_s{i}")
                mm(ps[:], neT[:, i * 128 : (i + 1) * 128], neT[:], start=True, stop=True)
                return ps

            # ---- cell stage emitters ----------------------------------------
            # vsb layout [128, 1024]: block j*256 -> [v1_j(128) | v2_j(128)]
            # vcsb layout [128, 512]: block j*128 -> [vc1_j(64) | vc2_j(64)]
            def _cp_op(eng, dst, src):
                if eng == "v":
                    nc.vector.tensor_copy(dst, src)
                else:
                    nc.scalar.copy(dst, src)

            def emit_vbuild(b, inp, gws, tag, engs=("v", "v")):
                vsb = wp.tile([128, 4 * 256], f32, name=f"vsb_{tag}{b}", tag="vsb", bufs=4)
                for h in range(2):
                    ps = acc_tile(f"vps_{tag}{b}_{h}")
                    for jj in range(2):
                        j = 2 * h + jj
                        mm(
                            ps[:, jj * 256 : (jj + 1) * 256],
                            inp[:, j * 128 : (j + 1) * 128],
                            gws[:],
                            start=True,
                            stop=True,
                            skip_group_check=True,
                        )
                    _cp_op(engs[h], rr(vsb[:, h * 512 : (h + 1) * 512]), ps[:])
                return vsb

            def emit_zr(b, inp, vsb, gw0, at, at2, tag):
                zps = acc_tile(f"zps_{tag}{b}")
                mm(zps[:], gw0[:], inp[:], start=True, stop=False)
                for j in range(NT):
                    mm(
                        zps[:],
                        vsb[:, j * 256 : j * 256 + 128],
                        at[j][:],
                        start=False,
                        stop=False,
                    )
                for j in range(NT):
                    mm(
                        zps[:],
                        vsb[:, j * 256 + 128 : (j + 1) * 256],
                        at2[j][:],
                        start=False,
                        stop=(j == NT - 1),
                    )
                return zps

            def emit_vcps(b, cand, uws, tag):
                # two [128,512] psum halves, j-blocks (0,1) and (2,3)
                phs = []
                for h in range(2):
                    ps = acc_tile(f"vcps_{tag}{b}_{h}")
                    for jj in range(2):
                        j = 2 * h + jj
                        mm(
                            ps[:, jj * 256 : (jj + 1) * 256],
                            cand[:, j * 128 : (j + 1) * 128],
                            uws[:],
                            start=True,
                            stop=True,
                            skip_group_check=True,
                        )
                    phs.append(ps)
                return phs

            def emit_vcbuild(b, cand, uws, tag, eng=None):
                phs = emit_vcps(b, cand, uws, tag)
                vcsb = wp.tile([128, 512], f32, name=f"vcsb_{tag}{b}", tag="vcsb", bufs=4)
                for h, ps in enumerate(phs):
                    src = ps.rearrange("p (j s c) -> p j s c", j=2, s=4, c=64)[
                        :, :, 0:2, :
                    ]
                    dst = vcsb[:, h * 256 : (h + 1) * 256].rearrange(
                        "p (j s c) -> p j s c", j=2, s=2, c=64
                    )
                    _cp_op(eng, rr(dst), src)
                return vcsb

            def emit_vc_pair_copy(x, phs, vcsb2, tag, eng=None):
                # pair layout per 384-block j:
                #   [vc1(b0)|vc1(b1)|vc2(b0)|vc2(b1)|w0c(b0)|w0c(b1)]
                for h, ps in enumerate(phs):
                    src = ps.rearrange("p (j s c) -> p j s c", j=2, s=4, c=64)[
                        :, :, 0:3, :
                    ]
                    dst = vcsb2[:, h * 768 : (h + 1) * 768].rearrange(
                        "p (j s x c) -> p j s x c", j=2, s=3, x=2, c=64
                    )[:, :, :, x, :]
                    _cp_op(eng, rr(dst), src)

            def emit_hc_pair(p_i, vcsb2, at, at2, tag):
                # two batches' hc in one [128,512] accumulation (enc only:
                # shared support). rows 0:64 = batch 2p, rows 64:128 = 2p+1.
                # The uW0 (identity-graph) term flows through the vc-build's
                # third block, applied against the identity slabs.
                hps = acc_tile(f"hps2_{tag}{p_i}")
                for j in range(NT):
                    mm(
                        hps[:],
                        vcsb2[:, j * 384 : j * 384 + 128],
                        at[j][:],
                        start=(j == 0), stop=False, skip_group_check=True,
                    )
                for j in range(NT):
                    mm(
                        hps[:],
                        vcsb2[:, j * 384 + 128 : j * 384 + 256],
                        at2[j][:],
                        start=False, stop=False, skip_group_check=True,
                    )
                for j in range(NT):
                    mm(
                        hps[:],
                        vcsb2[:, j * 384 + 256 : (j + 1) * 384],
                        idsl[j][:],
                        start=False, stop=(j == NT - 1), skip_group_check=True,
                    )
                return hps

            def emit_hc(b, cand, vcsb, uw0, at, at2, tag):
                hps = acc_tile(f"hps_{tag}{b}", p=HID)
                mm(hps[:], uw0[:], cand[:], start=True, stop=False)
                for j in range(NT):
                    mm(
                        hps[:],
                        vcsb[:, j * 128 : j * 128 + 64],
                        at[j][:],
                        start=False,
                        stop=False,
                    )
                for j in range(NT):
                    mm(
                        hps[:],
                        vcsb[:, j * 128 + 64 : (j + 1) * 128],
                        at2[j][:],
                        start=False,
                        stop=(j == NT - 1),
                    )
                return hps

            def emit_gate_z(b, zps, gbz, tag):
                zt = wp.tile([HID, N], f32, name=f"z_{tag}{b}", tag="zsb", bufs=4)
                nc.scalar.activation(zt[:], zps[0:HID, :], Act.Sigmoid, bias=gbz[:])
                return zt

            def emit_gate_r(b, zps, gbr, tag):
                rt = wp.tile([HID, N], f32, name=f"r_{tag}{b}", tag="rsb", bufs=4)
                nc.scalar.activation(
                    rt[:], zps[HID : 2 * HID, :], Act.Sigmoid, bias=gbr[:]
                )
                return rt

            def emit_cand_h(b, zt, inp, cand, tag):
                # alternate engines so 4 batches' zh don't serialize on Pool
                if b % 2 == 0:
                    nc.gpsimd.tensor_mul(rr(cand[0:HID, :]), zt[:], inp[0:HID, :])
                else:
                    nc.vector.tensor_mul(rr(cand[0:HID, :]), zt[:], inp[0:HID, :])

            def emit_update(b, rt, hps_ap, inp, nxt, ub, tag, do_add=True,
                            mul_eng="v"):
                # hct/dt feed the decoder proj matmuls -> rounded producers
                hct = wp.tile([HID, N], f32, name=f"hc_{tag}{b}", tag="hct", bufs=3)
                nc.scalar.activation(rr(hct[:]), hps_ap, Act.Tanh, bias=ub[:])
                dt = wp.tile([HID, N], f32, name=f"d_{tag}{b}", tag="dt", bufs=3)
                nc.gpsimd.tensor_sub(rr(dt[:]), inp[0:HID, :], hct[:])
                if mul_eng == "p":
                    # same-queue as the sub: dt ready without a cross-engine
                    # hop (the decoder proj-b matmul waits on it)
                    nc.gpsimd.tensor_mul(rr(dt[:]), rt[:], dt[:])
                else:
                    nc.vector.tensor_mul(rr(dt[:]), rt[:], dt[:])
                if do_add:
                    nc.vector.tensor_add(rr(nxt[0:HID, :]), hct[:], dt[:])
                return hct, dt

            # interleaved A/C phase: vb0 vb1 zr0 vb2 zr1 vb3 zr2 zr3.
            # post_z(b, zps) fires right after each zr group so the
            # z-sigmoid -> zh chain starts while later zr's still stream.
            def phase_AC(curs, gws, gw0, ats, at2s, tag, post_z=None, pre=None):
                vsbs = [None] * BL
                zps = [None] * BL
                order = [(0, "v"), (1, "v"), (2, "v"), (0, "z"), (3, "v"),
                        (1, "z"), (2, "z"), (3, "z")]
                for b, kind in order:
                    if kind == "v":
                        if pre is not None and pre[b] is not None:
                            vsbs[b] = pre[b]
                            continue
                        vsbs[b] = emit_vbuild(b, curs[b], gws, tag)
                    else:
                        zps[b] = emit_zr(
                            b, curs[b], vsbs[b], gw0, ats[b], at2s[b], tag
                        )
                        if post_z is not None:
                            post_z(b, zps[b])
                return vsbs, zps

            def phase_FH(cands, uws, uw0, ats, at2s, tag, cp_eng=None):
                vcsbs = [None] * BL
                hps = [None] * BL
                order = [(0, "v"), (1, "v"), (0, "h"), (2, "v"), (1, "h"), (3, "v"),
                        (2, "h"), (3, "h")]
                for b, kind in order:
                    if kind == "v":
                        vcsbs[b] = emit_vcbuild(b, cands[b], uws, tag, eng=cp_eng)
                    else:
                        hps[b] = emit_hc(
                            b, cands[b], vcsbs[b], uw0, ats[b], at2s[b], tag
                        )
                return vcsbs, hps

            def emit_FH_pair(pi, bs, uws, at, at2, tag, copy_eng):
                # FH for one batch pair: vcps x2, pair copies, joint hc accum
                ps_a = emit_vcps(bs[0], cand_t[bs[0]], uws, tag)
                ps_b = emit_vcps(bs[1], cand_t[bs[1]], uws, tag)
                v2 = wp.tile(
                    [128, 4 * 384], f32, name=f"vcsb2_{tag}p{pi}", tag="vsb", bufs=4
                )
                emit_vc_pair_copy(0, ps_a, v2, tag, copy_eng)
                emit_vc_pair_copy(1, ps_b, v2, tag, copy_eng)
                hp = emit_hc_pair(pi, v2, at, at2, tag)
                return hp[0:HID, :], hp[HID : 2 * HID, :]

            # ---- encoder ----------------------------------------------------
            pre_vsbs_e = [None] * BL

            def enc_mid():
                for b in range(BL):
                    pre_vsbs_e[b] = emit_vbuild(b, inp_pp[b][0], gws_e, "e0pre")

            build_supports([(enc_scores, aet, aet2)], "enc", mid=enc_mid)

            aets = [aet] * BL
            aet2s = [aet2] * BL
            for t in range(T):
                curs = [inp_pp[b][t % 2] for b in range(BL)]
                nxts = [inp_pp[b][(t + 1) % 2] for b in range(BL)]
                tag = f"e{t}"
                for b in range(BL):
                    if t + 1 < T:
                        rdma(nxts[b][HID:CIN, :], x_d[b, t + 1])
                    rdma(cand_t[b][HID:CIN, :], x_d[b, t])
                zts = [None] * BL

                def post_z_e(b, zp, zts=zts, curs=curs, tag=tag):
                    zts[b] = emit_gate_z(b, zp, gbz_e, tag)
                    emit_cand_h(b, zts[b], curs[b], cand_t[b], tag)

                vsbs, zps = phase_AC(
                    curs, gws_e, gw0_e, aets, aet2s, tag, post_z=post_z_e,
                    pre=(pre_vsbs_e if t == 0 else None),
                )
                # r-sigmoids only feed the update tail
                rts = [emit_gate_r(b, zps[b], gbr_e, tag) for b in range(BL)]
                vcsbs, hps = phase_FH(cand_t, uws_e, uw0_e, aets, aet2s, tag)
                for b in range(BL):
                    emit_update(
                        b, rts[b], hps[b][:], curs[b], nxts[b], ub_e, tag
                    )

            # ---- decoder supports (hyper-network), 2-way interleaved --------
            adt = [
                [cp.tile([128, N], f32, name=f"adt{b}_{j}") for j in range(NT)]
                for b in range(BL)
            ]
            adt2 = [
                [cp.tile([128, N], f32, name=f"adt2_{b}_{j}") for j in range(NT)]
                for b in range(BL)
            ]
            h_fin = [inp_pp[b][T % 2] for b in range(BL)]
            # go_0 = 0 and y_0 into the first cur + cand rows 64:66 (before
            # the support builds so the pre-emitted t=0 vbuilds can read them)
            for b in range(BL):
                cur0 = inp_pp[b][T % 2]
                nc.vector.tensor_copy(
                    rr(cur0[HID : HID + 1, :]), zrow[HID : HID + 1, :]
                )
                nc.vector.tensor_copy(
                    rr(cand_t[b][HID : HID + 1, :]), zrow[HID : HID + 1, :]
                )
                rdma(cur0[HID + 1 : CIN, :], y_d[b, 0])
                rdma(cand_t[b][HID + 1 : CIN, :], y_d[b, 0])

            pre_vsbs_d = [None] * BL

            def dec_mid():
                for b in range(BL):
                    pre_vsbs_d[b] = emit_vbuild(b, h_fin[b], gws_d, "d0pre")

            for g in range(2):
                builds = []
                for b in (2 * g, 2 * g + 1):
                    ps_h = acc_tile(f"ps_hyp{b}", p=EMB)
                    mm(ps_h[:], hwt[:], h_fin[b][0:HID, :], start=True, stop=True)
                    neb = wp.tile([EMB, N], f32, name=f"neb{b}", tag="neb", bufs=2)
                    nc.scalar.activation(rr(neb[:]), ps_h[:], Act.Identity, bias=hb[:])

                    def dec_scores(i, neb=neb, b=b):
                        ps = acc_tile(f"ps_dec_s{b}_{i}")
                        mm(
                            ps[:],
                            neb[:, i * 128 : (i + 1) * 128],
                            neb[:],
                            start=True,
                            stop=True,
                        )
                        return ps

                    builds.append((dec_scores, adt[b], adt2[b]))
                build_supports(
                    builds, f"dec{g}", mid=(dec_mid if g == 0 else None)
                )

            # ---- decoder ----------------------------------------------------
            adts = [adt[b] for b in range(BL)]
            adt2s = [adt2[b] for b in range(BL)]
            for t in range(T):
                curs = [inp_pp[b][(T + t) % 2] for b in range(BL)]
                nxts = [inp_pp[b][(T + t + 1) % 2] for b in range(BL)]
                tag = f"d{t}"
                zts = [None] * BL

                def post_z_d(b, zp, zts=zts, curs=curs, tag=tag):
                    zts[b] = emit_gate_z(b, zp, gbz_d, tag)
                    emit_cand_h(b, zts[b], curs[b], cand_t[b], tag)

                vsbs, zps = phase_AC(
                    curs, gws_d, gw0_d, adts, adt2s, tag, post_z=post_z_d,
                    pre=(pre_vsbs_d if t == 0 else None),
                )
                rts = [emit_gate_r(b, zps[b], gbr_d, tag) for b in range(BL)]
                vcsbs, hps = phase_FH(cand_t, uws_d, uw0_d, adts, adt2s, tag, cp_eng="v")
                # update tail split so the proj matmuls unblock early: per
                # batch tanh+sub, then all muls, then projs, then the h'
                # adds (only next step's vbuilds need those).
                hcts, dts = [], []
                for b in range(BL):
                    hct = wp.tile(
                        [HID, N], f32, name=f"hc_{tag}{b}", tag="hct", bufs=3
                    )
                    nc.scalar.activation(
                        rr(hct[:]), hps[b][:], Act.Tanh, bias=ub_d[:]
                    )
                    dt = wp.tile([HID, N], f32, name=f"d_{tag}{b}", tag="dt", bufs=3)
                    nc.gpsimd.tensor_sub(rr(dt[:]), curs[b][0:HID, :], hct[:])
                    hcts.append(hct)
                    dts.append(dt)
                for b in range(BL):
                    nc.vector.tensor_mul(rr(dts[b][:]), rts[b][:], dts[b][:])
                for b in range(BL):
                    # go_t straight into next step's cur rows 64:65 (+ cand via
                    # SBUF-SBUF DMA), y_{t+1} prefetched into rows 65:66.
                    # go = (hc + dt) @ pw computed as two accumulating matmuls
                    # so the proj doesn't wait on the final h' add.
                    psg = acc_tile(f"psg_{tag}{b}", p=1)
                    mm(psg[:], pw[:], hcts[b][:], start=True, stop=False)
                    mm(psg[:], pw[:], dts[b][:], start=False, stop=True)
                    nc.scalar.activation(
                        rr(nxts[b][HID : HID + 1, :]), psg[:], Act.Identity,
                        bias=pb[:],
                    )
                    nc.sync.dma_start(
                        out_d[b, t].rearrange("n c -> c n"),
                        nxts[b][HID : HID + 1, :],
                    )
                    if t + 1 < T:
                        rdma(
                            cand_t[b][HID : HID + 1, :], nxts[b][HID : HID + 1, :]
                        )
                        rdma(nxts[b][HID + 1 : CIN, :], y_d[b, t + 1])
                        rdma(cand_t[b][HID + 1 : CIN, :], y_d[b, t + 1])
                if t + 1 < T:
                    for b in range(BL):
                        nc.vector.tensor_add(
                            rr(nxts[b][0:HID, :]), hcts[b][:], dts[b][:]
                        )

    nc.compile()
    return nc


def _get_module():
    if "nc" not in _CACHE:
        _CACHE["nc"] = _build_module()
    return _CACHE["nc"]


def _in_maps(inputs):
    shared = {
        k: np.ascontiguousarray(np.asarray(inputs[k], dtype=np.float32))
        for k in (
            "node_emb",
            "enc_gW",
            "enc_gb",
            "enc_uW",
            "enc_ub",
            "dec_gW",
            "dec_gb",
            "dec_uW",
            "dec_ub",
            "proj_W",
            "proj_b",
            "hyper_W",
            "hyper_b",
        )
    }
    # pre-transpose to [B, T, C, N] so the kernel's per-step loads are
    # contiguous feature-major rows
    x = np.ascontiguousarray(
        np.asarray(inputs["x"], dtype=np.float32).transpose(0, 1, 3, 2)
    )
    y = np.ascontiguousarray(
        np.asarray(inputs["y_cov"], dtype=np.float32).transpose(0, 1, 3, 2)
    )
    maps = []
    for c in range(NCORES):
        m = dict(shared)
        m["x"] = np.ascontiguousarray(x[c * BL : (c + 1) * BL])
        m["y_cov"] = np.ascontiguousarray(y[c * BL : (c + 1) * BL])
        maps.append(m)
    return maps


def kernel(**inputs) -> np.ndarray:
    from concourse.bass_utils import run_bass_kernel_spmd

    nc = _get_module()
    maps = _in_maps(inputs)
    res = run_bass_kernel_spmd(nc, maps, list(range(NCORES)))
    out = np.concatenate([res.results[c]["out"] for c in range(NCORES)], axis=0)
    return out.astype(np.float32)


# revision 91
# speedup vs baseline: 1.0031x; 1.0031x over previous
"""DGCRN Trainium2 Bass kernel (restructured).

Problem: nn_DGCRN_67327907332247 (B=32, T=12, N=512, DIN=2, HID=64, CHEB_K=3,
EMB=10, DOUT=1, YCOV=1). Data-parallel over batch: 8 cores x 4 batches each.

Design (v2 — "v-projection" formulation, batch-stage interleaving):
 - State feature-major: inp [66,512] = [h(0:64); xt/go,ycov(64:66)].
 - Per gate path, exploit linearity G_k(x) W_k = (x W_k) pre-projected:
     v_j = inp_nm[j] @ [W1|W2]   (4 matmuls, moving dim 256, node-major out)
     zr^T = W0^T@inp + sum_j v1_j^T@A^T_j + sum_j v2_j^T@A2^T_j  (9 mm, 512)
   This kills all per-cell PE transposes and the xg PSUM->SBUF round trips.
 - z/r gates fused into one [128,512] accumulation + one sigmoid (bias is the
   stacked [gbz;gbr]).
 - Work for the 4 local batches is emitted stage-interleaved (vb0 vb1 zr0 vb2
   zr1 ...) so the PE never drains; a continuously-busy PE ramps from 1.2GHz
   to 2.4GHz (p-state) which alone is ~2x on matmul time.
 - All matmuls f32r with moving dims >= 256 (1 cycle/row). Producers of
   matmul operands write through .bitcast(f32r) APs (BIR rounded-producer
   rule); DMA loads are staged + round-copied.
 - Supports: A^T / (2A^2-I)^T tiles SBUF-resident as in v1; softmax skips
   row-max (relu-bounded); -I handled by subtracting the [128,128] identity
   from the diagonal block (no idsl slabs).
 - Elementwise work spread across Scalar(ACT)/Vector(DVE)/GpSimd(Pool) to
   stay under the PE's per-cell time.
"""

import numpy as np

B = 32
NCORES = 8
BL = B // NCORES  # 4 local batches
T = 12
N = 512
NT = N // 128  # 4 node tiles
DIN = 2
HID = 64
EMB = 10
CIN = DIN + HID  # 66
K = 3

_CACHE = {}


def _build_module():
    import concourse.bacc as bacc
    import concourse.mybir as mybir
    from concourse import masks, tile

    f32 = mybir.dt.float32
    Act = mybir.ActivationFunctionType
    f32r = mybir.dt.float32r

    nc = bacc.Bacc("TRN2", target_bir_lowering=False, debug=False)

    def mm(out, lhsT, rhs, **kw):
        nc.tensor.matmul(out, lhsT.bitcast(f32r), rhs.bitcast(f32r), **kw)

    def rr(ap):
        return ap.bitcast(f32r)

    # x/y are fed pre-transposed (feature-major rows) by _in_maps so the
    # per-step loads are contiguous DMAs instead of 512-way gathers.
    x_d = nc.dram_tensor("x", [BL, T, DIN, N], f32, kind="ExternalInput").ap()
    y_d = nc.dram_tensor("y_cov", [BL, T, 1, N], f32, kind="ExternalInput").ap()
    ne_d = nc.dram_tensor("node_emb", [N, EMB], f32, kind="ExternalInput").ap()
    egw_d = nc.dram_tensor("enc_gW", [K * CIN, 2 * HID], f32, kind="ExternalInput").ap()
    egb_d = nc.dram_tensor("enc_gb", [2 * HID], f32, kind="ExternalInput").ap()
    euw_d = nc.dram_tensor("enc_uW", [K * CIN, HID], f32, kind="ExternalInput").ap()
    eub_d = nc.dram_tensor("enc_ub", [HID], f32, kind="ExternalInput").ap()
    dgw_d = nc.dram_tensor("dec_gW", [K * CIN, 2 * HID], f32, kind="ExternalInput").ap()
    dgb_d = nc.dram_tensor("dec_gb", [2 * HID], f32, kind="ExternalInput").ap()
    duw_d = nc.dram_tensor("dec_uW", [K * CIN, HID], f32, kind="ExternalInput").ap()
    dub_d = nc.dram_tensor("dec_ub", [HID], f32, kind="ExternalInput").ap()
    pw_d = nc.dram_tensor("proj_W", [HID, 1], f32, kind="ExternalInput").ap()
    pb_d = nc.dram_tensor("proj_b", [1], f32, kind="ExternalInput").ap()
    hw_d = nc.dram_tensor("hyper_W", [HID, EMB], f32, kind="ExternalInput").ap()
    hb_d = nc.dram_tensor("hyper_b", [EMB], f32, kind="ExternalInput").ap()
    out_d = nc.dram_tensor("out", [BL, T, N, 1], f32, kind="ExternalOutput").ap()

    with tile.TileContext(nc) as tc:
        with (
            tc.tile_pool(name="const", bufs=1) as cp,
            tc.tile_pool(name="state", bufs=1) as sp,
            tc.tile_pool(name="work", bufs=2) as wp,
            tc.tile_pool(name="psum", bufs=1, space="PSUM") as pp,
        ):
            ident = cp.tile([128, 128], f32)
            masks.make_identity(nc, ident[:])

            def rdma(dst, src):
                # DMA whose dest AP is f32r-bitcast: satisfies the BIR
                # rounded-producer rule without a staging round-copy.
                nc.sync.dma_start(dst.bitcast(f32r), src.bitcast(f32r))

            # node-embedding + x0 loads first on the Sync queue: the encoder
            # support build and first cells (startup critical path) wait on
            # these; weight DMAs have more slack.
            ne_nm = []
            for i in range(NT):
                t = wp.tile([128, EMB], f32, name=f"ne_nm{i}", tag="ne_nm", bufs=NT)
                nc.sync.dma_start(t[:], ne_d[i * 128 : (i + 1) * 128, :])
                ne_nm.append(t)

            # ---- persistent per-batch state ---------------------------------
            zrow = wp.tile([CIN, N], f32, tag="wstg3")
            nc.vector.memset(zrow[:], 0.0)
            inp_pp = []
            cand_t = []
            for b in range(BL):
                pair = []
                for p in range(2):
                    it = sp.tile([CIN, N], f32, name=f"inp{b}_{p}")
                    if p == 0:
                        nc.vector.tensor_copy(rr(it[0:HID, :]), zrow[0:HID, :])
                    pair.append(it)
                inp_pp.append(pair)
                ct = sp.tile([CIN, N], f32, name=f"cand{b}")
                cand_t.append(ct)
            # prime xt for t=0 (DMA straight into state rows 64:66)
            for b in range(BL):
                rdma(inp_pp[b][0][HID:CIN, :], x_d[b, 0])

            def acc_tile(name, p=128, n=N):
                return pp.tile([p, n], f32, name=name, tag="acc", bufs=7)

            # ---- weights ----------------------------------------------------
            # feature layout [h(0:64); xt(64:66)] -> permute W rows.
            def wload(dst, src, k, c0, cols):
                rdma(dst[0:HID, c0 : c0 + cols], src[k * CIN + DIN : (k + 1) * CIN, :])
                rdma(dst[HID:CIN, c0 : c0 + cols], src[k * CIN : k * CIN + DIN, :])

            gw0_e = cp.tile([CIN, 2 * HID], f32)
            gws_e = cp.tile([CIN, 4 * HID], f32)
            uw0_e = cp.tile([CIN, HID], f32)
            uws_e = cp.tile([CIN, 4 * HID], f32)
            gw0_d = cp.tile([CIN, 2 * HID], f32)
            gws_d = cp.tile([CIN, 4 * HID], f32)
            uw0_d = cp.tile([CIN, HID], f32)
            uws_d = cp.tile([CIN, 4 * HID], f32)
            wload(gw0_e, egw_d, 0, 0, 2 * HID)
            wload(gws_e, egw_d, 1, 0, 2 * HID)
            wload(gws_e, egw_d, 2, 2 * HID, 2 * HID)
            # identity slabs: zeros except I at column block j (f32r-produced)
            zfull = wp.tile([128, N], f32, tag="wstg4")
            nc.vector.memset(zfull[:], 0.0)
            idsl = []
            for j in range(NT):
                s = cp.tile([128, N], f32, name=f"idsl{j}")
                nc.vector.tensor_copy(rr(s[:]), zfull[:])
                nc.vector.tensor_copy(rr(s[:, j * 128 : (j + 1) * 128]), ident[:])
                idsl.append(s)
            wload(uw0_e, euw_d, 0, 0, HID)
            wload(uws_e, euw_d, 1, 0, HID)
            wload(uws_e, euw_d, 2, HID, HID)
            wload(uws_e, euw_d, 0, 2 * HID, HID)  # uW0 via identity slabs (enc)
            wload(gw0_d, dgw_d, 0, 0, 2 * HID)
            wload(gws_d, dgw_d, 1, 0, 2 * HID)
            wload(gws_d, dgw_d, 2, 2 * HID, 2 * HID)
            wload(uw0_d, duw_d, 0, 0, HID)
            wload(uws_d, duw_d, 1, 0, HID)
            wload(uws_d, duw_d, 2, HID, HID)
            # zero-pad tails of the uW stacks (keeps moving dim 256)
            zpad = wp.tile([CIN, 2 * HID], f32, tag="wstg")
            nc.vector.memset(zpad[:], 0.0)
            nc.scalar.copy(rr(uws_e[:, 3 * HID : 4 * HID]), zpad[:, 0:HID])
            nc.scalar.copy(rr(uws_d[:, 2 * HID : 4 * HID]), zpad[:])

            def bias_tile(name, src, n):
                t = cp.tile([n, 1], f32, name=name)
                nc.sync.dma_start(t[:], src.rearrange("(p o) -> p o", o=1))
                return t

            gbz_e = bias_tile("gbz_e", egb_d[0:HID], HID)
            gbr_e = bias_tile("gbr_e", egb_d[HID : 2 * HID], HID)
            ub_e = bias_tile("ub_e", eub_d, HID)
            gbz_d = bias_tile("gbz_d", dgb_d[0:HID], HID)
            gbr_d = bias_tile("gbr_d", dgb_d[HID : 2 * HID], HID)
            ub_d = bias_tile("ub_d", dub_d, HID)
            pb = bias_tile("pb", pb_d, 1)
            hb = bias_tile("hb", hb_d, EMB)
            pw = cp.tile([HID, 1], f32)
            rdma(pw[:], pw_d[:, :])
            hwt = cp.tile([HID, EMB], f32)
            rdma(hwt[:], hw_d[:, :])

            # ---- support construction (n-way interleaved) -------------------
            # mid() is called between the softmax and transpose phases: the
            # PE idles there while DVE/Scalar finish the softmax chains, so
            # support-independent matmuls (the first timestep's vbuilds) can
            # fill the hole.
            def build_supports(builds, tag, mid=None):
                # builds: list of (emit_scores, at_tiles, at2_tiles)
                anm = {}
                for i in range(NT):
                    for bi, (es, _, _) in enumerate(builds):
                        ps = es(i)
                        nc.vector.tensor_scalar_max(ps[:], ps[:], 0.0)
                        ex = wp.tile(
                            [128, N], f32, name=f"ex_{tag}{bi}_{i}", tag="anm", bufs=8
                        )
                        esum = wp.tile(
                            [128, 1], f32, name=f"es_{tag}{bi}_{i}", tag="esum", bufs=4
                        )
                        nc.scalar.activation(
                            rr(ex[:]), ps[:], Act.Exp, accum_out=esum[:]
                        )
                        rinv = wp.tile(
                            [128, 1], f32, name=f"ri_{tag}{bi}_{i}", tag="rinv", bufs=4
                        )
                        nc.vector.reciprocal(rinv[:], esum[:])
                        nc.vector.tensor_scalar_mul(rr(ex[:]), ex[:], rinv[:])
                        anm[bi, i] = ex
                if mid is not None:
                    mid()
                for bi, (_, at, _) in enumerate(builds):
                    for j in range(NT):
                        ps_t = acc_tile(f"ps_t_{tag}{bi}_{j}")
                        for i in range(NT):
                            nc.tensor.matmul(
                                ps_t[:, i * 128 : (i + 1) * 128],
                                anm[bi, i][:, j * 128 : (j + 1) * 128],
                                ident[:],
                                is_transpose=True,
                                skip_group_check=True,
                            )
                        nc.scalar.copy(rr(at[j][:]), ps_t[:])
                for bi, (_, at, at2) in enumerate(builds):
                    for j in range(NT):
                        ps_c = acc_tile(f"ps_c_{tag}{bi}_{j}")
                        for k in range(NT):
                            mm(
                                ps_c[:],
                                anm[bi, k][:, j * 128 : (j + 1) * 128],
                                at[k][:],
                                start=(k == 0),
                                stop=(k == NT - 1),
                            )
                        nc.scalar.mul(rr(at2[j][:]), ps_c[:], 2.0)
                        nc.vector.tensor_sub(
                            rr(at2[j][:, j * 128 : (j + 1) * 128]),
                            at2[j][:, j * 128 : (j + 1) * 128],
                            ident[:],
                        )

            # ---- encoder support --------------------------------------------
            ps_ne = acc_tile("ps_ne", p=EMB)
            for i in range(NT):
                nc.tensor.matmul(
                    ps_ne[:, i * 128 : (i + 1) * 128],
                    ne_nm[i][:],
                    ident[:],
                    is_transpose=True,
                    skip_group_check=True,
                )
            neT = cp.tile([EMB, N], f32)
            nc.vector.tensor_copy(rr(neT[:]), ps_ne[:])

            aet = [cp.tile([128, N], f32, name=f"aet{j}") for j in range(NT)]
            aet2 = [cp.tile([128, N], f32, name=f"aet2_{j}") for j in range(NT)]

            def enc_scores(i):
                ps = acc_tile(f"ps_enc_s{i}")
                mm(ps[:], neT[:, i * 128 : (i + 1) * 128], neT[:], start=True, stop=True)
                return ps

            # ---- cell stage emitters ----------------------------------------
            # vsb layout [128, 1024]: block j*256 -> [v1_j(128) | v2_j(128)]
            # vcsb layout [128, 512]: block j*128 -> [vc1_j(64) | vc2_j(64)]
            def _cp_op(eng, dst, src):
                if eng == "v":
                    nc.vector.tensor_copy(dst, src)
                else:
                    nc.scalar.copy(dst, src)

            def emit_vbuild(b, inp, gws, tag, engs=("v", "v")):
                vsb = wp.tile([128, 4 * 256], f32, name=f"vsb_{tag}{b}", tag="vsb", bufs=4)
                for h in range(2):
                    ps = acc_tile(f"vps_{tag}{b}_{h}")
                    for jj in range(2):
                        j = 2 * h + jj
                        mm(
                            ps[:, jj * 256 : (jj + 1) * 256],
                            inp[:, j * 128 : (j + 1) * 128],
                            gws[:],
                            start=True,
                            stop=True,
                            skip_group_check=True,
                        )
                    _cp_op(engs[h], rr(vsb[:, h * 512 : (h + 1) * 512]), ps[:])
                return vsb

            def emit_zr(b, inp, vsb, gw0, at, at2, tag):
                zps = acc_tile(f"zps_{tag}{b}")
                mm(zps[:], gw0[:], inp[:], start=True, stop=False)
                for j in range(NT):
                    mm(
                        zps[:],
                        vsb[:, j * 256 : j * 256 + 128],
                        at[j][:],
                        start=False,
                        stop=False,
                    )
                for j in range(NT):
                    mm(
                        zps[:],
                        vsb[:, j * 256 + 128 : (j + 1) * 256],
                        at2[j][:],
                        start=False,
                        stop=(j == NT - 1),
                    )
                return zps

            def emit_vcps(b, cand, uws, tag):
                # two [128,512] psum halves, j-blocks (0,1) and (2,3)
                phs = []
                for h in range(2):
                    ps = acc_tile(f"vcps_{tag}{b}_{h}")
                    for jj in range(2):
                        j = 2 * h + jj
                        mm(
                            ps[:, jj * 256 : (jj + 1) * 256],
                            cand[:, j * 128 : (j + 1) * 128],
                            uws[:],
                            start=True,
                            stop=True,
                            skip_group_check=True,
                        )
                    phs.append(ps)
                return phs

            def emit_vcbuild(b, cand, uws, tag, eng=None):
                phs = emit_vcps(b, cand, uws, tag)
                vcsb = wp.tile([128, 512], f32, name=f"vcsb_{tag}{b}", tag="vcsb", bufs=4)
                for h, ps in enumerate(phs):
                    src = ps.rearrange("p (j s c) -> p j s c", j=2, s=4, c=64)[
                        :, :, 0:2, :
                    ]
                    dst = vcsb[:, h * 256 : (h + 1) * 256].rearrange(
                        "p (j s c) -> p j s c", j=2, s=2, c=64
                    )
                    _cp_op(eng, rr(dst), src)
                return vcsb

            def emit_vc_pair_copy(x, phs, vcsb2, tag, eng=None):
                # pair layout per 384-block j:
                #   [vc1(b0)|vc1(b1)|vc2(b0)|vc2(b1)|w0c(b0)|w0c(b1)]
                for h, ps in enumerate(phs):
                    src = ps.rearrange("p (j s c) -> p j s c", j=2, s=4, c=64)[
                        :, :, 0:3, :
                    ]
                    dst = vcsb2[:, h * 768 : (h + 1) * 768].rearrange(
                        "p (j s x c) -> p j s x c", j=2, s=3, x=2, c=64
                    )[:, :, :, x, :]
                    _cp_op(eng, rr(dst), src)

            def emit_hc_pair(p_i, vcsb2, at, at2, tag):
                # two batches' hc in one [128,512] accumulation (enc only:
                # shared support). rows 0:64 = batch 2p, rows 64:128 = 2p+1.
                # The uW0 (identity-graph) term flows through the vc-build's
                # third block, applied against the identity slabs.
                hps = acc_tile(f"hps2_{tag}{p_i}")
                for j in range(NT):
                    mm(
                        hps[:],
                        vcsb2[:, j * 384 : j * 384 + 128],
                        at[j][:],
                        start=(j == 0), stop=False, skip_group_check=True,
                    )
                for j in range(NT):
                    mm(
                        hps[:],
                        vcsb2[:, j * 384 + 128 : j * 384 + 256],
                        at2[j][:],
                        start=False, stop=False, skip_group_check=True,
                    )
                for j in range(NT):
                    mm(
                        hps[:],
                        vcsb2[:, j * 384 + 256 : (j + 1) * 384],
                        idsl[j][:],
                        start=False, stop=(j == NT - 1), skip_group_check=True,
                    )
                return hps

            def emit_hc(b, cand, vcsb, uw0, at, at2, tag):
                hps = acc_tile(f"hps_{tag}{b}", p=HID)
                mm(hps[:], uw0[:], cand[:], start=True, stop=False)
                for j in range(NT):
                    mm(
                        hps[:],
                        vcsb[:, j * 128 : j * 128 + 64],
                        at[j][:],
                        start=False,
                        stop=False,
                    )
                for j in range(NT):
                    mm(
                        hps[:],
                        vcsb[:, j * 128 + 64 : (j + 1) * 128],
                        at2[j][:],
                        start=False,
                        stop=(j == NT - 1),
                    )
                return hps

            def emit_gate_z(b, zps, gbz, tag):
                zt = wp.tile([HID, N], f32, name=f"z_{tag}{b}", tag="zsb", bufs=4)
                nc.scalar.activation(zt[:], zps[0:HID, :], Act.Sigmoid, bias=gbz[:])
                return zt

            def emit_gate_r(b, zps, gbr, tag):
                rt = wp.tile([HID, N], f32, name=f"r_{tag}{b}", tag="rsb", bufs=4)
                nc.scalar.activation(
                    rt[:], zps[HID : 2 * HID, :], Act.Sigmoid, bias=gbr[:]
                )
                return rt

            def emit_cand_h(b, zt, inp, cand, tag):
                # alternate engines so 4 batches' zh don't serialize on Pool
                if b % 2 == 0:
                    nc.gpsimd.tensor_mul(rr(cand[0:HID, :]), zt[:], inp[0:HID, :])
                else:
                    nc.vector.tensor_mul(rr(cand[0:HID, :]), zt[:], inp[0:HID, :])

            def emit_update(b, rt, hps_ap, inp, nxt, ub, tag, do_add=True,
                            mul_eng="v"):
                # hct/dt feed the decoder proj matmuls -> rounded producers
                hct = wp.tile([HID, N], f32, name=f"hc_{tag}{b}", tag="hct", bufs=3)
                nc.scalar.activation(rr(hct[:]), hps_ap, Act.Tanh, bias=ub[:])
                dt = wp.tile([HID, N], f32, name=f"d_{tag}{b}", tag="dt", bufs=3)
                nc.gpsimd.tensor_sub(rr(dt[:]), inp[0:HID, :], hct[:])
                if mul_eng == "p":
                    # same-queue as the sub: dt ready without a cross-engine
                    # hop (the decoder proj-b matmul waits on it)
                    nc.gpsimd.tensor_mul(rr(dt[:]), rt[:], dt[:])
                else:
                    nc.vector.tensor_mul(rr(dt[:]), rt[:], dt[:])
                if do_add:
                    nc.vector.tensor_add(rr(nxt[0:HID, :]), hct[:], dt[:])
                return hct, dt

            # interleaved A/C phase: vb0 vb1 zr0 vb2 zr1 vb3 zr2 zr3.
            # post_z(b, zps) fires right after each zr group so the
            # z-sigmoid -> zh chain starts while later zr's still stream.
            def phase_AC(curs, gws, gw0, ats, at2s, tag, post_z=None, pre=None):
                vsbs = [None] * BL
                zps = [None] * BL
                order = [(0, "v"), (1, "v"), (2, "v"), (0, "z"), (3, "v"),
                        (1, "z"), (2, "z"), (3, "z")]
                for b, kind in order:
                    if kind == "v":
                        if pre is not None and pre[b] is not None:
                            vsbs[b] = pre[b]
                            continue
                        vsbs[b] = emit_vbuild(b, curs[b], gws, tag)
                    else:
                        zps[b] = emit_zr(
                            b, curs[b], vsbs[b], gw0, ats[b], at2s[b], tag
                        )
                        if post_z is not None:
                            post_z(b, zps[b])
                return vsbs, zps

            def phase_FH(cands, uws, uw0, ats, at2s, tag, cp_eng=None):
                vcsbs = [None] * BL
                hps = [None] * BL
                order = [(0, "v"), (1, "v"), (0, "h"), (2, "v"), (1, "h"), (3, "v"),
                        (2, "h"), (3, "h")]
                for b, kind in order:
                    if kind == "v":
                        vcsbs[b] = emit_vcbuild(b, cands[b], uws, tag, eng=cp_eng)
                    else:
                        hps[b] = emit_hc(
                            b, cands[b], vcsbs[b], uw0, ats[b], at2s[b], tag
                        )
                return vcsbs, hps

            def emit_FH_pair(pi, bs, uws, at, at2, tag, copy_eng):
                # FH for one batch pair: vcps x2, pair copies, joint hc accum
                ps_a = emit_vcps(bs[0], cand_t[bs[0]], uws, tag)
                ps_b = emit_vcps(bs[1], cand_t[bs[1]], uws, tag)
                v2 = wp.tile(
                    [128, 4 * 384], f32, name=f"vcsb2_{tag}p{pi}", tag="vsb", bufs=4
                )
                emit_vc_pair_copy(0, ps_a, v2, tag, copy_eng)
                emit_vc_pair_copy(1, ps_b, v2, tag, copy_eng)
                hp = emit_hc_pair(pi, v2, at, at2, tag)
                return hp[0:HID, :], hp[HID : 2 * HID, :]

            # ---- encoder ----------------------------------------------------
            pre_vsbs_e = [None] * BL

            def enc_mid():
                for b in range(BL):
                    pre_vsbs_e[b] = emit_vbuild(b, inp_pp[b][0], gws_e, "e0pre")

            build_supports([(enc_scores, aet, aet2)], "enc")

            aets = [aet] * BL
            aet2s = [aet2] * BL
            for t in range(T):
                curs = [inp_pp[b][t % 2] for b in range(BL)]
                nxts = [inp_pp[b][(t + 1) % 2] for b in range(BL)]
                tag = f"e{t}"
                for b in range(BL):
                    if t + 1 < T:
                        rdma(nxts[b][HID:CIN, :], x_d[b, t + 1])
                    rdma(cand_t[b][HID:CIN, :], x_d[b, t])
                zts = [None] * BL

                def post_z_e(b, zp, zts=zts, curs=curs, tag=tag):
                    zts[b] = emit_gate_z(b, zp, gbz_e, tag)
                    emit_cand_h(b, zts[b], curs[b], cand_t[b], tag)

                vsbs, zps = phase_AC(
                    curs, gws_e, gw0_e, aets, aet2s, tag, post_z=post_z_e,
                    pre=(pre_vsbs_e if t == 0 else None),
                )
                # r-sigmoids only feed the update tail
                rts = [emit_gate_r(b, zps[b], gbr_e, tag) for b in range(BL)]
                vcsbs, hps = phase_FH(cand_t, uws_e, uw0_e, aets, aet2s, tag)
                for b in range(BL):
                    emit_update(
                        b, rts[b], hps[b][:], curs[b], nxts[b], ub_e, tag
                    )

            # ---- decoder supports (hyper-network), 2-way interleaved --------
            adt = [
                [cp.tile([128, N], f32, name=f"adt{b}_{j}") for j in range(NT)]
                for b in range(BL)
            ]
            adt2 = [
                [cp.tile([128, N], f32, name=f"adt2_{b}_{j}") for j in range(NT)]
                for b in range(BL)
            ]
            h_fin = [inp_pp[b][T % 2] for b in range(BL)]
            # go_0 = 0 and y_0 into the first cur + cand rows 64:66 (before
            # the support builds so the pre-emitted t=0 vbuilds can read them)
            for b in range(BL):
                cur0 = inp_pp[b][T % 2]
                nc.vector.tensor_copy(
                    rr(cur0[HID : HID + 1, :]), zrow[HID : HID + 1, :]
                )
                nc.vector.tensor_copy(
                    rr(cand_t[b][HID : HID + 1, :]), zrow[HID : HID + 1, :]
                )
                rdma(cur0[HID + 1 : CIN, :], y_d[b, 0])
                rdma(cand_t[b][HID + 1 : CIN, :], y_d[b, 0])

            pre_vsbs_d = [None] * BL

            def dec_mid():
                for b in range(BL):
                    pre_vsbs_d[b] = emit_vbuild(b, h_fin[b], gws_d, "d0pre")

            for g in range(2):
                builds = []
                for b in (2 * g, 2 * g + 1):
                    ps_h = acc_tile(f"ps_hyp{b}", p=EMB)
                    mm(ps_h[:], hwt[:], h_fin[b][0:HID, :], start=True, stop=True)
                    neb = wp.tile([EMB, N], f32, name=f"neb{b}", tag="neb", bufs=2)
                    nc.scalar.activation(rr(neb[:]), ps_h[:], Act.Identity, bias=hb[:])

                    def dec_scores(i, neb=neb, b=b):
                        ps = acc_tile(f"ps_dec_s{b}_{i}")
                        mm(
                            ps[:],
                            neb[:, i * 128 : (i + 1) * 128],
                            neb[:],
                            start=True,
                            stop=True,
                        )
                        return ps

                    builds.append((dec_scores, adt[b], adt2[b]))
                build_supports(builds, f"dec{g}")

            # ---- decoder ----------------------------------------------------
            adts = [adt[b] for b in range(BL)]
            adt2s = [adt2[b] for b in range(BL)]
            for t in range(T):
                curs = [inp_pp[b][(T + t) % 2] for b in range(BL)]
                nxts = [inp_pp[b][(T + t + 1) % 2] for b in range(BL)]
                tag = f"d{t}"
                zts = [None] * BL

                def post_z_d(b, zp, zts=zts, curs=curs, tag=tag):
                    zts[b] = emit_gate_z(b, zp, gbz_d, tag)
                    emit_cand_h(b, zts[b], curs[b], cand_t[b], tag)

                vsbs, zps = phase_AC(
                    curs, gws_d, gw0_d, adts, adt2s, tag, post_z=post_z_d,
                    pre=(pre_vsbs_d if t == 0 else None),
                )
                rts = [emit_gate_r(b, zps[b], gbr_d, tag) for b in range(BL)]
                vcsbs, hps = phase_FH(cand_t, uws_d, uw0_d, adts, adt2s, tag, cp_eng="v")
                # update tail split so the proj matmuls unblock early: per
                # batch tanh+sub, then all muls, then projs, then the h'
                # adds (only next step's vbuilds need those).
                hcts, dts = [], []
                for b in range(BL):
                    hct = wp.tile(
                        [HID, N], f32, name=f"hc_{tag}{b}", tag="hct", bufs=3
                    )
                    nc.scalar.activation(
                        rr(hct[:]), hps[b][:], Act.Tanh, bias=ub_d[:]
                    )
                    dt = wp.tile([HID, N], f32, name=f"d_{tag}{b}", tag="dt", bufs=3)
                    nc.gpsimd.tensor_sub(rr(dt[:]), curs[b][0:HID, :], hct[:])
                    hcts.append(hct)
                    dts.append(dt)
                for b in range(BL):
                    nc.vector.tensor_mul(rr(dts[b][:]), rts[b][:], dts[b][:])
                for b in range(BL):
                    # go_t straight into next step's cur rows 64:65 (+ cand via
                    # SBUF-SBUF DMA), y_{t+1} prefetched into rows 65:66.
                    # go = (hc + dt) @ pw computed as two accumulating matmuls
                    # so the proj doesn't wait on the final h' add.
                    psg = acc_tile(f"psg_{tag}{b}", p=1)
                    mm(psg[:], pw[:], hcts[b][:], start=True, stop=False)
                    mm(psg[:], pw[:], dts[b][:], start=False, stop=True)
                    nc.scalar.activation(
                        rr(nxts[b][HID : HID + 1, :]), psg[:], Act.Identity,
                        bias=pb[:],
                    )
                    nc.sync.dma_start(
                        out_d[b, t].rearrange("n c -> c n"),
                        nxts[b][HID : HID + 1, :],
                    )
                    if t + 1 < T:
                        rdma(
                            cand_t[b][HID : HID + 1, :], nxts[b][HID : HID + 1, :]
                        )
                        rdma(nxts[b][HID + 1 : CIN, :], y_d[b, t + 1])
                        rdma(cand_t[b][HID + 1 : CIN, :], y_d[b, t + 1])
                if t + 1 < T:
                    for b in range(BL):
                        nc.vector.tensor_add(
                            rr(nxts[b][0:HID, :]), hcts[b][:], dts[b][:]
                        )

    nc.compile()
    return nc


def _get_module():
    if "nc" not in _CACHE:
        _CACHE["nc"] = _build_module()
    return _CACHE["nc"]


def _in_maps(inputs):
    shared = {
        k: np.ascontiguousarray(np.asarray(inputs[k], dtype=np.float32))
        for k in (
            "node_emb",
            "enc_gW",
            "enc_gb",
            "enc_uW",
            "enc_ub",
            "dec_gW",
            "dec_gb",
            "dec_uW",
            "dec_ub",
            "proj_W",
            "proj_b",
            "hyper_W",
            "hyper_b",
        )
    }
    # pre-transpose to [B, T, C, N] so the kernel's per-step loads are
    # contiguous feature-major rows
    x = np.ascontiguousarray(
        np.asarray(inputs["x"], dtype=np.float32).transpose(0, 1, 3, 2)
    )
    y = np.ascontiguousarray(
        np.asarray(inputs["y_cov"], dtype=np.float32).transpose(0, 1, 3, 2)
    )
    maps = []
    for c in range(NCORES):
        m = dict(shared)
        m["x"] = np.ascontiguousarray(x[c * BL : (c + 1) * BL])
        m["y_cov"] = np.ascontiguousarray(y[c * BL : (c + 1) * BL])
        maps.append(m)
    return maps


def kernel(**inputs) -> np.ndarray:
    from concourse.bass_utils import run_bass_kernel_spmd

    nc = _get_module()
    maps = _in_maps(inputs)
    res = run_bass_kernel_spmd(nc, maps, list(range(NCORES)))
    out = np.concatenate([res.results[c]["out"] for c in range(NCORES)], axis=0)
    return out.astype(np.float32)


# revision 92
# speedup vs baseline: 1.0076x; 1.0044x over previous
"""DGCRN Trainium2 Bass kernel (restructured).

Problem: nn_DGCRN_67327907332247 (B=32, T=12, N=512, DIN=2, HID=64, CHEB_K=3,
EMB=10, DOUT=1, YCOV=1). Data-parallel over batch: 8 cores x 4 batches each.

Design (v2 — "v-projection" formulation, batch-stage interleaving):
 - State feature-major: inp [66,512] = [h(0:64); xt/go,ycov(64:66)].
 - Per gate path, exploit linearity G_k(x) W_k = (x W_k) pre-projected:
     v_j = inp_nm[j] @ [W1|W2]   (4 matmuls, moving dim 256, node-major out)
     zr^T = W0^T@inp + sum_j v1_j^T@A^T_j + sum_j v2_j^T@A2^T_j  (9 mm, 512)
   This kills all per-cell PE transposes and the xg PSUM->SBUF round trips.
 - z/r gates fused into one [128,512] accumulation + one sigmoid (bias is the
   stacked [gbz;gbr]).
 - Work for the 4 local batches is emitted stage-interleaved (vb0 vb1 zr0 vb2
   zr1 ...) so the PE never drains; a continuously-busy PE ramps from 1.2GHz
   to 2.4GHz (p-state) which alone is ~2x on matmul time.
 - All matmuls f32r with moving dims >= 256 (1 cycle/row). Producers of
   matmul operands write through .bitcast(f32r) APs (BIR rounded-producer
   rule); DMA loads are staged + round-copied.
 - Supports: A^T / (2A^2-I)^T tiles SBUF-resident as in v1; softmax skips
   row-max (relu-bounded); -I handled by subtracting the [128,128] identity
   from the diagonal block (no idsl slabs).
 - Elementwise work spread across Scalar(ACT)/Vector(DVE)/GpSimd(Pool) to
   stay under the PE's per-cell time.
"""

import numpy as np

B = 32
NCORES = 8
BL = B // NCORES  # 4 local batches
T = 12
N = 512
NT = N // 128  # 4 node tiles
DIN = 2
HID = 64
EMB = 10
CIN = DIN + HID  # 66
K = 3

_CACHE = {}


def _build_module():
    import concourse.bacc as bacc
    import concourse.mybir as mybir
    from concourse import masks, tile

    f32 = mybir.dt.float32
    Act = mybir.ActivationFunctionType
    f32r = mybir.dt.float32r

    nc = bacc.Bacc("TRN2", target_bir_lowering=False, debug=False)

    def mm(out, lhsT, rhs, **kw):
        nc.tensor.matmul(out, lhsT.bitcast(f32r), rhs.bitcast(f32r), **kw)

    def rr(ap):
        return ap.bitcast(f32r)

    # x/y are fed pre-transposed (feature-major rows) by _in_maps so the
    # per-step loads are contiguous DMAs instead of 512-way gathers.
    x_d = nc.dram_tensor("x", [BL, T, DIN, N], f32, kind="ExternalInput").ap()
    y_d = nc.dram_tensor("y_cov", [BL, T, 1, N], f32, kind="ExternalInput").ap()
    ne_d = nc.dram_tensor("node_emb", [N, EMB], f32, kind="ExternalInput").ap()
    egw_d = nc.dram_tensor("enc_gW", [K * CIN, 2 * HID], f32, kind="ExternalInput").ap()
    egb_d = nc.dram_tensor("enc_gb", [2 * HID], f32, kind="ExternalInput").ap()
    euw_d = nc.dram_tensor("enc_uW", [K * CIN, HID], f32, kind="ExternalInput").ap()
    eub_d = nc.dram_tensor("enc_ub", [HID], f32, kind="ExternalInput").ap()
    dgw_d = nc.dram_tensor("dec_gW", [K * CIN, 2 * HID], f32, kind="ExternalInput").ap()
    dgb_d = nc.dram_tensor("dec_gb", [2 * HID], f32, kind="ExternalInput").ap()
    duw_d = nc.dram_tensor("dec_uW", [K * CIN, HID], f32, kind="ExternalInput").ap()
    dub_d = nc.dram_tensor("dec_ub", [HID], f32, kind="ExternalInput").ap()
    pw_d = nc.dram_tensor("proj_W", [HID, 1], f32, kind="ExternalInput").ap()
    pb_d = nc.dram_tensor("proj_b", [1], f32, kind="ExternalInput").ap()
    hw_d = nc.dram_tensor("hyper_W", [HID, EMB], f32, kind="ExternalInput").ap()
    hb_d = nc.dram_tensor("hyper_b", [EMB], f32, kind="ExternalInput").ap()
    out_d = nc.dram_tensor("out", [BL, T, N, 1], f32, kind="ExternalOutput").ap()

    with tile.TileContext(nc) as tc:
        with (
            tc.tile_pool(name="const", bufs=1) as cp,
            tc.tile_pool(name="state", bufs=1) as sp,
            tc.tile_pool(name="work", bufs=2) as wp,
            tc.tile_pool(name="psum", bufs=1, space="PSUM") as pp,
        ):
            ident = cp.tile([128, 128], f32)
            masks.make_identity(nc, ident[:])

            def rdma(dst, src):
                # DMA whose dest AP is f32r-bitcast: satisfies the BIR
                # rounded-producer rule without a staging round-copy.
                nc.sync.dma_start(dst.bitcast(f32r), src.bitcast(f32r))

            # node-embedding + x0 loads first on the Sync queue: the encoder
            # support build and first cells (startup critical path) wait on
            # these; weight DMAs have more slack.
            ne_nm = []
            for i in range(NT):
                t = wp.tile([128, EMB], f32, name=f"ne_nm{i}", tag="ne_nm", bufs=NT)
                nc.sync.dma_start(t[:], ne_d[i * 128 : (i + 1) * 128, :])
                ne_nm.append(t)

            # ---- persistent per-batch state ---------------------------------
            zrow = wp.tile([CIN, N], f32, tag="wstg3")
            nc.vector.memset(zrow[:], 0.0)
            inp_pp = []
            cand_t = []
            for b in range(BL):
                pair = []
                for p in range(2):
                    it = sp.tile([CIN, N], f32, name=f"inp{b}_{p}")
                    if p == 0:
                        nc.vector.tensor_copy(rr(it[0:HID, :]), zrow[0:HID, :])
                    pair.append(it)
                inp_pp.append(pair)
                ct = sp.tile([CIN, N], f32, name=f"cand{b}")
                cand_t.append(ct)
            # prime xt for t=0 (DMA straight into state rows 64:66)
            for b in range(BL):
                rdma(inp_pp[b][0][HID:CIN, :], x_d[b, 0])

            def acc_tile(name, p=128, n=N):
                return pp.tile([p, n], f32, name=name, tag="acc", bufs=8)

            # ---- weights ----------------------------------------------------
            # feature layout [h(0:64); xt(64:66)] -> permute W rows.
            def wload(dst, src, k, c0, cols):
                rdma(dst[0:HID, c0 : c0 + cols], src[k * CIN + DIN : (k + 1) * CIN, :])
                rdma(dst[HID:CIN, c0 : c0 + cols], src[k * CIN : k * CIN + DIN, :])

            gw0_e = cp.tile([CIN, 2 * HID], f32)
            gws_e = cp.tile([CIN, 4 * HID], f32)
            uw0_e = cp.tile([CIN, HID], f32)
            uws_e = cp.tile([CIN, 4 * HID], f32)
            gw0_d = cp.tile([CIN, 2 * HID], f32)
            gws_d = cp.tile([CIN, 4 * HID], f32)
            uw0_d = cp.tile([CIN, HID], f32)
            uws_d = cp.tile([CIN, 4 * HID], f32)
            wload(gw0_e, egw_d, 0, 0, 2 * HID)
            wload(gws_e, egw_d, 1, 0, 2 * HID)
            wload(gws_e, egw_d, 2, 2 * HID, 2 * HID)
            # identity slabs: zeros except I at column block j (f32r-produced)
            zfull = wp.tile([128, N], f32, tag="wstg4")
            nc.vector.memset(zfull[:], 0.0)
            idsl = []
            for j in range(NT):
                s = cp.tile([128, N], f32, name=f"idsl{j}")
                nc.vector.tensor_copy(rr(s[:]), zfull[:])
                nc.vector.tensor_copy(rr(s[:, j * 128 : (j + 1) * 128]), ident[:])
                idsl.append(s)
            wload(uw0_e, euw_d, 0, 0, HID)
            wload(uws_e, euw_d, 1, 0, HID)
            wload(uws_e, euw_d, 2, HID, HID)
            wload(uws_e, euw_d, 0, 2 * HID, HID)  # uW0 via identity slabs (enc)
            wload(gw0_d, dgw_d, 0, 0, 2 * HID)
            wload(gws_d, dgw_d, 1, 0, 2 * HID)
            wload(gws_d, dgw_d, 2, 2 * HID, 2 * HID)
            wload(uw0_d, duw_d, 0, 0, HID)
            wload(uws_d, duw_d, 1, 0, HID)
            wload(uws_d, duw_d, 2, HID, HID)
            # zero-pad tails of the uW stacks (keeps moving dim 256)
            zpad = wp.tile([CIN, 2 * HID], f32, tag="wstg")
            nc.vector.memset(zpad[:], 0.0)
            nc.scalar.copy(rr(uws_e[:, 3 * HID : 4 * HID]), zpad[:, 0:HID])
            nc.scalar.copy(rr(uws_d[:, 2 * HID : 4 * HID]), zpad[:])

            def bias_tile(name, src, n):
                t = cp.tile([n, 1], f32, name=name)
                nc.sync.dma_start(t[:], src.rearrange("(p o) -> p o", o=1))
                return t

            gbz_e = bias_tile("gbz_e", egb_d[0:HID], HID)
            gbr_e = bias_tile("gbr_e", egb_d[HID : 2 * HID], HID)
            ub_e = bias_tile("ub_e", eub_d, HID)
            gbz_d = bias_tile("gbz_d", dgb_d[0:HID], HID)
            gbr_d = bias_tile("gbr_d", dgb_d[HID : 2 * HID], HID)
            ub_d = bias_tile("ub_d", dub_d, HID)
            pb = bias_tile("pb", pb_d, 1)
            hb = bias_tile("hb", hb_d, EMB)
            pw = cp.tile([HID, 1], f32)
            rdma(pw[:], pw_d[:, :])
            hwt = cp.tile([HID, EMB], f32)
            rdma(hwt[:], hw_d[:, :])

            # ---- support construction (n-way interleaved) -------------------
            # mid() is called between the softmax and transpose phases: the
            # PE idles there while DVE/Scalar finish the softmax chains, so
            # support-independent matmuls (the first timestep's vbuilds) can
            # fill the hole.
            def build_supports(builds, tag, mid=None):
                # builds: list of (emit_scores, at_tiles, at2_tiles)
                anm = {}
                for i in range(NT):
                    for bi, (es, _, _) in enumerate(builds):
                        ps = es(i)
                        nc.vector.tensor_scalar_max(ps[:], ps[:], 0.0)
                        ex = wp.tile(
                            [128, N], f32, name=f"ex_{tag}{bi}_{i}", tag="anm", bufs=8
                        )
                        esum = wp.tile(
                            [128, 1], f32, name=f"es_{tag}{bi}_{i}", tag="esum", bufs=4
                        )
                        nc.scalar.activation(
                            rr(ex[:]), ps[:], Act.Exp, accum_out=esum[:]
                        )
                        rinv = wp.tile(
                            [128, 1], f32, name=f"ri_{tag}{bi}_{i}", tag="rinv", bufs=4
                        )
                        nc.vector.reciprocal(rinv[:], esum[:])
                        nc.vector.tensor_scalar_mul(rr(ex[:]), ex[:], rinv[:])
                        anm[bi, i] = ex
                if mid is not None:
                    mid()
                for bi, (_, at, _) in enumerate(builds):
                    for j in range(NT):
                        ps_t = acc_tile(f"ps_t_{tag}{bi}_{j}")
                        for i in range(NT):
                            nc.tensor.matmul(
                                ps_t[:, i * 128 : (i + 1) * 128],
                                anm[bi, i][:, j * 128 : (j + 1) * 128],
                                ident[:],
                                is_transpose=True,
                                skip_group_check=True,
                            )
                        nc.scalar.copy(rr(at[j][:]), ps_t[:])
                for bi, (_, at, at2) in enumerate(builds):
                    for j in range(NT):
                        ps_c = acc_tile(f"ps_c_{tag}{bi}_{j}")
                        for k in range(NT):
                            mm(
                                ps_c[:],
                                anm[bi, k][:, j * 128 : (j + 1) * 128],
                                at[k][:],
                                start=(k == 0),
                                stop=(k == NT - 1),
                            )
                        nc.scalar.mul(rr(at2[j][:]), ps_c[:], 2.0)
                        nc.vector.tensor_sub(
                            rr(at2[j][:, j * 128 : (j + 1) * 128]),
                            at2[j][:, j * 128 : (j + 1) * 128],
                            ident[:],
                        )

            # ---- encoder support --------------------------------------------
            ps_ne = acc_tile("ps_ne", p=EMB)
            for i in range(NT):
                nc.tensor.matmul(
                    ps_ne[:, i * 128 : (i + 1) * 128],
                    ne_nm[i][:],
                    ident[:],
                    is_transpose=True,
                    skip_group_check=True,
                )
            neT = cp.tile([EMB, N], f32)
            nc.vector.tensor_copy(rr(neT[:]), ps_ne[:])

            aet = [cp.tile([128, N], f32, name=f"aet{j}") for j in range(NT)]
            aet2 = [cp.tile([128, N], f32, name=f"aet2_{j}") for j in range(NT)]

            def enc_scores(i):
                ps = acc_tile(f"ps_enc_s{i}")
                mm(ps[:], neT[:, i * 128 : (i + 1) * 128], neT[:], start=True, stop=True)
                return ps

            # ---- cell stage emitters ----------------------------------------
            # vsb layout [128, 1024]: block j*256 -> [v1_j(128) | v2_j(128)]
            # vcsb layout [128, 512]: block j*128 -> [vc1_j(64) | vc2_j(64)]
            def _cp_op(eng, dst, src):
                if eng == "v":
                    nc.vector.tensor_copy(dst, src)
                else:
                    nc.scalar.copy(dst, src)

            def emit_vbuild(b, inp, gws, tag, engs=("v", "v")):
                vsb = wp.tile([128, 4 * 256], f32, name=f"vsb_{tag}{b}", tag="vsb", bufs=4)
                for h in range(2):
                    ps = acc_tile(f"vps_{tag}{b}_{h}")
                    for jj in range(2):
                        j = 2 * h + jj
                        mm(
                            ps[:, jj * 256 : (jj + 1) * 256],
                            inp[:, j * 128 : (j + 1) * 128],
                            gws[:],
                            start=True,
                            stop=True,
                            skip_group_check=True,
                        )
                    _cp_op(engs[h], rr(vsb[:, h * 512 : (h + 1) * 512]), ps[:])
                return vsb

            def emit_zr(b, inp, vsb, gw0, at, at2, tag):
                zps = acc_tile(f"zps_{tag}{b}")
                mm(zps[:], gw0[:], inp[:], start=True, stop=False)
                for j in range(NT):
                    mm(
                        zps[:],
                        vsb[:, j * 256 : j * 256 + 128],
                        at[j][:],
                        start=False,
                        stop=False,
                    )
                for j in range(NT):
                    mm(
                        zps[:],
                        vsb[:, j * 256 + 128 : (j + 1) * 256],
                        at2[j][:],
                        start=False,
                        stop=(j == NT - 1),
                    )
                return zps

            def emit_vcps(b, cand, uws, tag):
                # two [128,512] psum halves, j-blocks (0,1) and (2,3)
                phs = []
                for h in range(2):
                    ps = acc_tile(f"vcps_{tag}{b}_{h}")
                    for jj in range(2):
                        j = 2 * h + jj
                        mm(
                            ps[:, jj * 256 : (jj + 1) * 256],
                            cand[:, j * 128 : (j + 1) * 128],
                            uws[:],
                            start=True,
                            stop=True,
                            skip_group_check=True,
                        )
                    phs.append(ps)
                return phs

            def emit_vcbuild(b, cand, uws, tag, eng=None):
                phs = emit_vcps(b, cand, uws, tag)
                vcsb = wp.tile([128, 512], f32, name=f"vcsb_{tag}{b}", tag="vcsb", bufs=4)
                for h, ps in enumerate(phs):
                    src = ps.rearrange("p (j s c) -> p j s c", j=2, s=4, c=64)[
                        :, :, 0:2, :
                    ]
                    dst = vcsb[:, h * 256 : (h + 1) * 256].rearrange(
                        "p (j s c) -> p j s c", j=2, s=2, c=64
                    )
                    _cp_op(eng, rr(dst), src)
                return vcsb

            def emit_vc_pair_copy(x, phs, vcsb2, tag, eng=None):
                # pair layout per 384-block j:
                #   [vc1(b0)|vc1(b1)|vc2(b0)|vc2(b1)|w0c(b0)|w0c(b1)]
                for h, ps in enumerate(phs):
                    src = ps.rearrange("p (j s c) -> p j s c", j=2, s=4, c=64)[
                        :, :, 0:3, :
                    ]
                    dst = vcsb2[:, h * 768 : (h + 1) * 768].rearrange(
                        "p (j s x c) -> p j s x c", j=2, s=3, x=2, c=64
                    )[:, :, :, x, :]
                    _cp_op(eng, rr(dst), src)

            def emit_hc_pair(p_i, vcsb2, at, at2, tag):
                # two batches' hc in one [128,512] accumulation (enc only:
                # shared support). rows 0:64 = batch 2p, rows 64:128 = 2p+1.
                # The uW0 (identity-graph) term flows through the vc-build's
                # third block, applied against the identity slabs.
                hps = acc_tile(f"hps2_{tag}{p_i}")
                for j in range(NT):
                    mm(
                        hps[:],
                        vcsb2[:, j * 384 : j * 384 + 128],
                        at[j][:],
                        start=(j == 0), stop=False, skip_group_check=True,
                    )
                for j in range(NT):
                    mm(
                        hps[:],
                        vcsb2[:, j * 384 + 128 : j * 384 + 256],
                        at2[j][:],
                        start=False, stop=False, skip_group_check=True,
                    )
                for j in range(NT):
                    mm(
                        hps[:],
                        vcsb2[:, j * 384 + 256 : (j + 1) * 384],
                        idsl[j][:],
                        start=False, stop=(j == NT - 1), skip_group_check=True,
                    )
                return hps

            def emit_hc(b, cand, vcsb, uw0, at, at2, tag):
                hps = acc_tile(f"hps_{tag}{b}", p=HID)
                mm(hps[:], uw0[:], cand[:], start=True, stop=False)
                for j in range(NT):
                    mm(
                        hps[:],
                        vcsb[:, j * 128 : j * 128 + 64],
                        at[j][:],
                        start=False,
                        stop=False,
                    )
                for j in range(NT):
                    mm(
                        hps[:],
                        vcsb[:, j * 128 + 64 : (j + 1) * 128],
                        at2[j][:],
                        start=False,
                        stop=(j == NT - 1),
                    )
                return hps

            def emit_gate_z(b, zps, gbz, tag):
                zt = wp.tile([HID, N], f32, name=f"z_{tag}{b}", tag="zsb", bufs=4)
                nc.scalar.activation(zt[:], zps[0:HID, :], Act.Sigmoid, bias=gbz[:])
                return zt

            def emit_gate_r(b, zps, gbr, tag):
                rt = wp.tile([HID, N], f32, name=f"r_{tag}{b}", tag="rsb", bufs=4)
                nc.scalar.activation(
                    rt[:], zps[HID : 2 * HID, :], Act.Sigmoid, bias=gbr[:]
                )
                return rt

            def emit_cand_h(b, zt, inp, cand, tag):
                # alternate engines so 4 batches' zh don't serialize on Pool
                if b % 2 == 0:
                    nc.gpsimd.tensor_mul(rr(cand[0:HID, :]), zt[:], inp[0:HID, :])
                else:
                    nc.vector.tensor_mul(rr(cand[0:HID, :]), zt[:], inp[0:HID, :])

            def emit_update(b, rt, hps_ap, inp, nxt, ub, tag, do_add=True,
                            mul_eng="v"):
                # hct/dt feed the decoder proj matmuls -> rounded producers
                hct = wp.tile([HID, N], f32, name=f"hc_{tag}{b}", tag="hct", bufs=3)
                nc.scalar.activation(rr(hct[:]), hps_ap, Act.Tanh, bias=ub[:])
                dt = wp.tile([HID, N], f32, name=f"d_{tag}{b}", tag="dt", bufs=3)
                nc.gpsimd.tensor_sub(rr(dt[:]), inp[0:HID, :], hct[:])
                if mul_eng == "p":
                    # same-queue as the sub: dt ready without a cross-engine
                    # hop (the decoder proj-b matmul waits on it)
                    nc.gpsimd.tensor_mul(rr(dt[:]), rt[:], dt[:])
                else:
                    nc.vector.tensor_mul(rr(dt[:]), rt[:], dt[:])
                if do_add:
                    nc.vector.tensor_add(rr(nxt[0:HID, :]), hct[:], dt[:])
                return hct, dt

            # interleaved A/C phase: vb0 vb1 zr0 vb2 zr1 vb3 zr2 zr3.
            # post_z(b, zps) fires right after each zr group so the
            # z-sigmoid -> zh chain starts while later zr's still stream.
            def phase_AC(curs, gws, gw0, ats, at2s, tag, post_z=None, pre=None):
                vsbs = [None] * BL
                zps = [None] * BL
                order = [(0, "v"), (1, "v"), (2, "v"), (0, "z"), (3, "v"),
                        (1, "z"), (2, "z"), (3, "z")]
                for b, kind in order:
                    if kind == "v":
                        if pre is not None and pre[b] is not None:
                            vsbs[b] = pre[b]
                            continue
                        vsbs[b] = emit_vbuild(b, curs[b], gws, tag)
                    else:
                        zps[b] = emit_zr(
                            b, curs[b], vsbs[b], gw0, ats[b], at2s[b], tag
                        )
                        if post_z is not None:
                            post_z(b, zps[b])
                return vsbs, zps

            def phase_FH(cands, uws, uw0, ats, at2s, tag, cp_eng=None):
                vcsbs = [None] * BL
                hps = [None] * BL
                order = [(0, "v"), (1, "v"), (0, "h"), (2, "v"), (1, "h"), (3, "v"),
                        (2, "h"), (3, "h")]
                for b, kind in order:
                    if kind == "v":
                        vcsbs[b] = emit_vcbuild(b, cands[b], uws, tag, eng=cp_eng)
                    else:
                        hps[b] = emit_hc(
                            b, cands[b], vcsbs[b], uw0, ats[b], at2s[b], tag
                        )
                return vcsbs, hps

            def emit_FH_pair(pi, bs, uws, at, at2, tag, copy_eng):
                # FH for one batch pair: vcps x2, pair copies, joint hc accum
                ps_a = emit_vcps(bs[0], cand_t[bs[0]], uws, tag)
                ps_b = emit_vcps(bs[1], cand_t[bs[1]], uws, tag)
                v2 = wp.tile(
                    [128, 4 * 384], f32, name=f"vcsb2_{tag}p{pi}", tag="vsb", bufs=4
                )
                emit_vc_pair_copy(0, ps_a, v2, tag, copy_eng)
                emit_vc_pair_copy(1, ps_b, v2, tag, copy_eng)
                hp = emit_hc_pair(pi, v2, at, at2, tag)
                return hp[0:HID, :], hp[HID : 2 * HID, :]

            # ---- encoder ----------------------------------------------------
            pre_vsbs_e = [None] * BL

            def enc_mid():
                for b in range(BL):
                    pre_vsbs_e[b] = emit_vbuild(b, inp_pp[b][0], gws_e, "e0pre")

            build_supports([(enc_scores, aet, aet2)], "enc")

            aets = [aet] * BL
            aet2s = [aet2] * BL
            for t in range(T):
                curs = [inp_pp[b][t % 2] for b in range(BL)]
                nxts = [inp_pp[b][(t + 1) % 2] for b in range(BL)]
                tag = f"e{t}"
                for b in range(BL):
                    if t + 1 < T:
                        rdma(nxts[b][HID:CIN, :], x_d[b, t + 1])
                    rdma(cand_t[b][HID:CIN, :], x_d[b, t])
                zts = [None] * BL

                def post_z_e(b, zp, zts=zts, curs=curs, tag=tag):
                    zts[b] = emit_gate_z(b, zp, gbz_e, tag)
                    emit_cand_h(b, zts[b], curs[b], cand_t[b], tag)

                vsbs, zps = phase_AC(
                    curs, gws_e, gw0_e, aets, aet2s, tag, post_z=post_z_e,
                    pre=(pre_vsbs_e if t == 0 else None),
                )
                # r-sigmoids only feed the update tail
                rts = [emit_gate_r(b, zps[b], gbr_e, tag) for b in range(BL)]
                vcsbs, hps = phase_FH(cand_t, uws_e, uw0_e, aets, aet2s, tag)
                for b in range(BL):
                    emit_update(
                        b, rts[b], hps[b][:], curs[b], nxts[b], ub_e, tag
                    )

            # ---- decoder supports (hyper-network), 2-way interleaved --------
            adt = [
                [cp.tile([128, N], f32, name=f"adt{b}_{j}") for j in range(NT)]
                for b in range(BL)
            ]
            adt2 = [
                [cp.tile([128, N], f32, name=f"adt2_{b}_{j}") for j in range(NT)]
                for b in range(BL)
            ]
            h_fin = [inp_pp[b][T % 2] for b in range(BL)]
            # go_0 = 0 and y_0 into the first cur + cand rows 64:66 (before
            # the support builds so the pre-emitted t=0 vbuilds can read them)
            for b in range(BL):
                cur0 = inp_pp[b][T % 2]
                nc.vector.tensor_copy(
                    rr(cur0[HID : HID + 1, :]), zrow[HID : HID + 1, :]
                )
                nc.vector.tensor_copy(
                    rr(cand_t[b][HID : HID + 1, :]), zrow[HID : HID + 1, :]
                )
                rdma(cur0[HID + 1 : CIN, :], y_d[b, 0])
                rdma(cand_t[b][HID + 1 : CIN, :], y_d[b, 0])

            pre_vsbs_d = [None] * BL

            def dec_mid():
                for b in range(BL):
                    pre_vsbs_d[b] = emit_vbuild(b, h_fin[b], gws_d, "d0pre")

            for g in range(2):
                builds = []
                for b in (2 * g, 2 * g + 1):
                    ps_h = acc_tile(f"ps_hyp{b}", p=EMB)
                    mm(ps_h[:], hwt[:], h_fin[b][0:HID, :], start=True, stop=True)
                    neb = wp.tile([EMB, N], f32, name=f"neb{b}", tag="neb", bufs=2)
                    nc.scalar.activation(rr(neb[:]), ps_h[:], Act.Identity, bias=hb[:])

                    def dec_scores(i, neb=neb, b=b):
                        ps = acc_tile(f"ps_dec_s{b}_{i}")
                        mm(
                            ps[:],
                            neb[:, i * 128 : (i + 1) * 128],
                            neb[:],
                            start=True,
                            stop=True,
                        )
                        return ps

                    builds.append((dec_scores, adt[b], adt2[b]))
                build_supports(builds, f"dec{g}")

            # ---- decoder ----------------------------------------------------
            adts = [adt[b] for b in range(BL)]
            adt2s = [adt2[b] for b in range(BL)]
            for t in range(T):
                curs = [inp_pp[b][(T + t) % 2] for b in range(BL)]
                nxts = [inp_pp[b][(T + t + 1) % 2] for b in range(BL)]
                tag = f"d{t}"
                zts = [None] * BL

                def post_z_d(b, zp, zts=zts, curs=curs, tag=tag):
                    zts[b] = emit_gate_z(b, zp, gbz_d, tag)
                    emit_cand_h(b, zts[b], curs[b], cand_t[b], tag)

                vsbs, zps = phase_AC(
                    curs, gws_d, gw0_d, adts, adt2s, tag, post_z=post_z_d,
                    pre=(pre_vsbs_d if t == 0 else None),
                )
                rts = [emit_gate_r(b, zps[b], gbr_d, tag) for b in range(BL)]
                vcsbs, hps = phase_FH(cand_t, uws_d, uw0_d, adts, adt2s, tag, cp_eng="v")
                # update tail split so the proj matmuls unblock early: per
                # batch tanh+sub, then all muls, then projs, then the h'
                # adds (only next step's vbuilds need those).
                hcts, dts = [], []
                for b in range(BL):
                    hct = wp.tile(
                        [HID, N], f32, name=f"hc_{tag}{b}", tag="hct", bufs=3
                    )
                    nc.scalar.activation(
                        rr(hct[:]), hps[b][:], Act.Tanh, bias=ub_d[:]
                    )
                    dt = wp.tile([HID, N], f32, name=f"d_{tag}{b}", tag="dt", bufs=3)
                    nc.gpsimd.tensor_sub(rr(dt[:]), curs[b][0:HID, :], hct[:])
                    hcts.append(hct)
                    dts.append(dt)
                for b in range(BL):
                    nc.vector.tensor_mul(rr(dts[b][:]), rts[b][:], dts[b][:])
                for b in range(BL):
                    # go_t straight into next step's cur rows 64:65 (+ cand via
                    # SBUF-SBUF DMA), y_{t+1} prefetched into rows 65:66.
                    # go = (hc + dt) @ pw computed as two accumulating matmuls
                    # so the proj doesn't wait on the final h' add.
                    psg = acc_tile(f"psg_{tag}{b}", p=1)
                    mm(psg[:], pw[:], hcts[b][:], start=True, stop=False)
                    mm(psg[:], pw[:], dts[b][:], start=False, stop=True)
                    nc.scalar.activation(
                        rr(nxts[b][HID : HID + 1, :]), psg[:], Act.Identity,
                        bias=pb[:],
                    )
                    nc.sync.dma_start(
                        out_d[b, t].rearrange("n c -> c n"),
                        nxts[b][HID : HID + 1, :],
                    )
                    if t + 1 < T:
                        rdma(
                            cand_t[b][HID : HID + 1, :], nxts[b][HID : HID + 1, :]
                        )
                        rdma(nxts[b][HID + 1 : CIN, :], y_d[b, t + 1])
                        rdma(cand_t[b][HID + 1 : CIN, :], y_d[b, t + 1])
                if t + 1 < T:
                    for b in range(BL):
                        nc.vector.tensor_add(
                            rr(nxts[b][0:HID, :]), hcts[b][:], dts[b][:]
                        )

    nc.compile()
    return nc


def _get_module():
    if "nc" not in _CACHE:
        _CACHE["nc"] = _build_module()
    return _CACHE["nc"]


def _in_maps(inputs):
    shared = {
        k: np.ascontiguousarray(np.asarray(inputs[k], dtype=np.float32))
        for k in (
            "node_emb",
            "enc_gW",
            "enc_gb",
            "enc_uW",
            "enc_ub",
            "dec_gW",
            "dec_gb",
            "dec_uW",
            "dec_ub",
            "proj_W",
            "proj_b",
            "hyper_W",
            "hyper_b",
        )
    }
    # pre-transpose to [B, T, C, N] so the kernel's per-step loads are
    # contiguous feature-major rows
    x = np.ascontiguousarray(
        np.asarray(inputs["x"], dtype=np.float32).transpose(0, 1, 3, 2)
    )
    y = np.ascontiguousarray(
        np.asarray(inputs["y_cov"], dtype=np.float32).transpose(0, 1, 3, 2)
    )
    maps = []
    for c in range(NCORES):
        m = dict(shared)
        m["x"] = np.ascontiguousarray(x[c * BL : (c + 1) * BL])
        m["y_cov"] = np.ascontiguousarray(y[c * BL : (c + 1) * BL])
        maps.append(m)
    return maps


def kernel(**inputs) -> np.ndarray:
    from concourse.bass_utils import run_bass_kernel_spmd

    nc = _get_module()
    maps = _in_maps(inputs)
    res = run_bass_kernel_spmd(nc, maps, list(range(NCORES)))
    out = np.concatenate([res.results[c]["out"] for c in range(NCORES)], axis=0)
    return out.astype(np.float32)
